# revision 25
# baseline (speedup 1.0000x reference)
"""Trainium2 kernel for nn_CP1_17669495456474 (sparse_attention).
8-core data-parallel: core = (sample, spatial half). Device computes the
grouped cross-correlation (per core: 2016 f-positions x 1024 kernels,
K=1024 contraction) as fp16 tensor-engine matmuls. Contraction is split
as partition=(d, c) with dy=2*dy1+d, so the dy1/dx kernel offsets are
free-dim AP shifts. The kernel rhs slices are pre-arranged contiguously
on host (Kn [128,16,512]) so the PE streams without strided SBUF reads;
G is a single unreplicated [128,34,66] tile sliced per matmul. Host
applies the cheap fuse/mask/softmax."""
import sys, types
import numpy as np

import concourse.bass as bass
import concourse.mybir as mybir
from concourse.tile import TileContext
import concourse.tile as tile_mod
import concourse.bass_utils as bass_utils

F16 = mybir.dt.float16
F32 = mybir.dt.float32
AOT = mybir.AluOpType
NT, TP, L = 16, 126, 1024

# ---------------- compile workarounds (walrus sync-wait limits) ----------------
import orjson

def _patched_drain_and_barrier(self, tick_clock, wait_clock):
    nc = self.nc
    ScopedClock = tile_mod.ScopedClock
    drain_inst = nc.sync.drain()
    wait_clock.add_sem_waits(drain_inst.ins, ScopedClock({None: tick_clock.global_clock}))
    waits = list(drain_inst.ins.sync_info.on_wait)
    if len(waits) > 1:
        import bass_rust
        drain_inst.ins.sync_info = bass_rust.SyncInfo(on_wait=waits[:1], on_update=[])
        for i in range(1, len(waits)):
            d2 = nc.sync.drain()
            d2.ins.sync_info = bass_rust.SyncInfo(on_wait=[waits[i]], on_update=[])
    # No barrier, no sem clearing: every run_bass_kernel_spmd call is a
    # fresh NEFF load (sems re-initialized), and the sync drain above
    # already waits for all engine work + DMA completions, so other
    # engines can halt as soon as their streams end. The full teardown
    # (barrier butterfly + gpsimd dma_reset + 2nd barrier) costs ~3us.
    popped = nc._tile_sem_poison_stack.pop()
    assert popped is self._sem_poison

def _dedup_ldweights(m):
    # Drop Ldweights identical to the weights already resident in the PE
    # array (consecutive same-weight loads, e.g. the n=0/n=1 matmul pairs).
    # Sync carried by a dropped load moves to the next PE instruction.
    for f in m.get("functions", []):
        for b in f.get("blocks", []):
            out = []
            last_sig = None
            carry_w, carry_u = [], []
            for inst in b.get("instructions", []):
                if inst.get("engine") == "PE":
                    opc = inst.get("opcode", "")
                    if opc == "Ldweights":
                        sig = orjson.dumps([inst.get("ins"),
                                            inst.get("tile_position"),
                                            inst.get("tile_size"),
                                            inst.get("perf_mode"),
                                            inst.get("is_transpose")])
                        if sig == last_sig:
                            si = inst.get("sync_info") or {}
                            carry_w.extend(si.get("on_wait") or [])
                            carry_u.extend(si.get("on_update") or [])
                            continue
                        last_sig = sig
                    elif opc != "Matmult":
                        last_sig = None
                    if carry_w or carry_u:
                        si = inst.get("sync_info")
                        if si is None:
                            si = {"on_update": [], "on_wait": []}
                            inst["sync_info"] = si
                        si["on_wait"] = carry_w + (si.get("on_wait") or [])
                        si["on_update"] = (si.get("on_update") or []) + carry_u
                        carry_w, carry_u = [], []
                out.append(inst)
            b["instructions"] = out
    return m

def _split_waits_json(bir_bytes):
    m = _dedup_ldweights(orjson.loads(bir_bytes))
    for f in m.get("functions", []):
        for b in f.get("blocks", []):
            insts = b.get("instructions", [])
            out = []
            for inst in insts:
                si = inst.get("sync_info")
                waits = (si or {}).get("on_wait") or []
                opc = inst.get("opcode", "")
                is_dma = opc.startswith("DMA") or "Trigger" in opc or "Dma" in opc
                keep = 1
                if is_dma and len(waits) <= 1:
                    out.append(inst)
                    continue
                if len(waits) > keep:
                    si["on_wait"] = waits[-keep:]
                    for i, w in enumerate(waits[:-keep]):
                        out.append({
                            "debug": inst.get("debug", 0), "engine": inst["engine"],
                            "ins": [], "outs": [], "name": f"{inst['name']}_xw{i}",
                            "opcode": "EventSemaphore",
                            "sync_info": {"on_update": [], "on_wait": [w]},
                        })
                out.append(inst)
            b["instructions"] = out
    return orjson.dumps(m)

def _install_patches():
    if getattr(bass_utils.compile_bir_kernel, "_wait_split", False):
        return
    TileContext._drain_and_barrier = _patched_drain_and_barrier
    import concourse.bass2jax as b2j
    orig = bass_utils.compile_bir_kernel
    def wrapped(bir_str, *a, **kw):
        if isinstance(bir_str, (bytes, bytearray)):
            try:
                bir_str = _split_waits_json(bir_str)
            except Exception:
                pass
        return orig(bir_str, *a, **kw)
    wrapped._wait_split = True
    bass_utils.compile_bir_kernel = wrapped
    if hasattr(b2j, "compile_bir_kernel"):
        b2j.compile_bir_kernel = wrapped
    # NTFF hook shim so trace=True doesn't crash if requested elsewhere
    if "antenv.axon_hooks" not in sys.modules:
        mod = types.ModuleType("antenv.axon_hooks")
        mod._hook = None
        mod.set_axon_ntff_profile_hook = lambda h: setattr(mod, "_hook", h)
        mod.get_axon_ntff_profile_hook = lambda: mod._hook
        sys.modules["antenv.axon_hooks"] = mod
        try:
            from trn_agent_boot.trn_boot import _ntff_profile_via_ctypes
            hk = _ntff_profile_via_ctypes('/opt/axon/libaxon_pjrt.so')
            if hk is not None:
                mod._hook = hk
        except Exception:
            pass
        bass_utils.upload_artifacts = lambda tmpdir: str(tmpdir)

# ---------------- device program: raw cos in [p, l] tiles ----------------
_NC_CACHE = [None]

def _build_nc():
    if _NC_CACHE[0] is not None:
        return _NC_CACHE[0]
    _install_patches()
    nc = bass.Bass("TRN2", target_bir_lowering=False, debug=False)
    g_d = nc.dram_tensor("g", [128, 34, 66], F16, kind="ExternalInput")
    K_d = nc.dram_tensor("kn", [128, 65, 66], F16, kind="ExternalInput")
    o_d = nc.dram_tensor("o", [NT, TP, L], F16, kind="ExternalOutput")
    with TileContext(nc) as tc:
        import contextlib
        ctx = contextlib.ExitStack()
        with ctx:
            const = ctx.enter_context(tc.tile_pool(name="const", bufs=1))
            outp = ctx.enter_context(tc.tile_pool(name="outp", bufs=3))
            psp = ctx.enter_context(tc.tile_pool(name="psp", bufs=3, space="PSUM"))
            psw = ctx.enter_context(tc.tile_pool(name="psw", bufs=1, space="PSUM"))
            G1 = const.tile([128, 34, 66], F16, tag="G1")
            G4 = const.tile([128, 4, 34, 63], F16, tag="G4")
            Ks = const.tile([128, 65, 66], F16, tag="Ks")
            Kn = const.tile([128, 16, 512], F16, tag="Kn")
            Wd = const.tile([128, 126], F16, tag="Wd")
            Xd = const.tile([128, 512], F16, tag="Xd")
            # warmup: ramp the PE p-state while the inputs land
            nc.vector.memset(Wd[:], 0.0)
            nc.gpsimd.memset(Xd[:], 0.0)
            pw = psw.tile([128, 512], F32, tag="pw")
            for _ in range(14):
                nc.tensor.matmul(pw[0:TP, :], Wd[:], Xd[:], start=True, stop=True,
                                 skip_group_check=True)
            # Input waves: both operands load COMPACT (G1 574KB + Ks 1.07MB
            # instead of 4.45MB replicated) and are rearranged on-device by
            # the otherwise-idle vector/scalar/gpsimd engines:
            #   G4[:, dx, y, :] = G1[:, y, dx:dx+63]        (dx replication)
            #   Kn[:, 8n+4dy1+dx] = Ks[:, 32n+2dy1::2, dx::2] (contiguous rhs)
            engs = [nc.scalar, nc.sync, nc.gpsimd]
            q = 0
            wave = [(G1[:, 0:8], g_d[:, 0:8]),
                    (Ks[:, 0:12], K_d[:, 0:12]),
                    (Ks[:, 12:23], K_d[:, 12:23]),
                    (Ks[:, 23:34], K_d[:, 23:34]),
                    (Ks[:, 34:45], K_d[:, 34:45]),
                    (Ks[:, 45:55], K_d[:, 45:55]),
                    (Ks[:, 55:65], K_d[:, 55:65]),
                    (G1[:, 8:21], g_d[:, 8:21]),
                    (G1[:, 21:34], g_d[:, 21:34])]
            for dst, src in wave:
                engs[q % 3].dma_start(out=dst, in_=src)
                q += 1

            def _copy(e, dst, src):
                if e is nc.scalar:
                    e.copy(out=dst, in_=src)
                else:
                    e.tensor_copy(dst, src)
            # G4 rows 0:8 first (tile 0's weights), then the 16 Kn slices in
            # consumption order (tile 0 is n-first), then the rest of G4.
            for dx in range(4):
                _copy((nc.vector, nc.gpsimd, nc.vector, nc.scalar)[dx],
                      G4[:, dx, 0:8, :], G1[:, 0:8, dx:dx+63])
            kceng = [nc.vector, nc.gpsimd, nc.scalar]
            for m in range(16):
                n, dy1, dx = m // 8, (m // 4) % 2, m % 4
                a = 32*n + 2*dy1
                _copy(kceng[m % 3], Kn[:, m], Ks[:, a:a+31:2, dx:dx+63:2])
            for ci, ys in enumerate((slice(8, 21), slice(21, 34))):
                for dx in range(4):
                    _copy((nc.vector, nc.gpsimd, nc.vector, nc.scalar)[dx],
                          G4[:, dx, ys, :], G1[:, ys, dx:dx+63])

            for t in range(NT):
                ps0 = psp.tile([128, 512], F32, tag="ps0", name="ps0")
                ps1 = psp.tile([128, 512], F32, tag="ps1", name="ps1")
                pss = (ps0, ps1)
                if t == 0 or t == NT - 1:
                    # n-first. Tile 0: the first 8 matmuls need only Kn[0:8],
                    # which lands earlier. Last tile: the n=0 PSUM bank
                    # finishes at matmul 8, letting its copy+DMA overlap the
                    # final 8 matmuls.
                    order = [(dy1, dx, n) for n in range(2) for dy1 in range(2)
                             for dx in range(4)]
                    starts = {0, 8}
                    stops = {7, 15}
                else:
                    order = [(dy1, dx, n) for dy1 in range(2) for dx in range(4)
                             for n in range(2)]
                    starts = {0, 1}
                    stops = {14, 15}
                for kk, (dy1, dx, n) in enumerate(order):
                    y0 = 2*t + 2*dy1
                    lhsT = G4[:, dx, y0:y0+2, :]
                    rhs = Kn[:, 8*n + 4*dy1 + dx]
                    nc.tensor.matmul(pss[n][0:TP, :], lhsT, rhs,
                                     start=(kk in starts), stop=(kk in stops),
                                     skip_group_check=True)
                O = outp.tile([128, 1024], F16, tag="O", name="O")
                if t == NT - 1:
                    # chunked copies + prompt DMA so the n=0 half streams out
                    # while the n=1 matmuls still run; keep gpsimd out of the
                    # final transfers so its (slowest) teardown drain starts
                    # as early as possible
                    nc.scalar.copy(out=O[0:TP, 0:256], in_=ps0[0:TP, 0:256])
                    nc.vector.tensor_copy(O[0:TP, 256:512], ps0[0:TP, 256:512])
                    nc.scalar.dma_start(out=o_d[t, :, 0:256], in_=O[0:TP, 0:256])
                    nc.sync.dma_start(out=o_d[t, :, 256:512], in_=O[0:TP, 256:512])
                    nc.scalar.copy(out=O[0:TP, 512:768], in_=ps1[0:TP, 0:256])
                    nc.vector.tensor_copy(O[0:TP, 768:1024], ps1[0:TP, 256:512])
                    nc.scalar.dma_start(out=o_d[t, :, 512:768], in_=O[0:TP, 512:768])
                    nc.sync.dma_start(out=o_d[t, :, 768:1024], in_=O[0:TP, 768:1024])
                else:
                    nc.scalar.copy(out=O[0:TP, 0:512], in_=ps0[0:TP, :])
                    nc.vector.tensor_copy(O[0:TP, 512:1024], ps1[0:TP, :])
                    e0, e1 = (nc.sync, nc.gpsimd) if t % 2 == 0 else (nc.gpsimd, nc.sync)
                    e0.dma_start(out=o_d[t, :, 0:512], in_=O[0:TP, 0:512])
                    e1.dma_start(out=o_d[t, :, 512:1024], in_=O[0:TP, 512:1024])
    _NC_CACHE[0] = nc
    return nc

# ---------------- host side ----------------
def _pad_edge3(x):
    return np.pad(x, ((0, 0), (1, 1), (1, 1)), mode='edge')

def _build_K(bnpad16):
    # [128, 65, 66]: K[(d,c), y, x] = bnpad[c, y+d, x]  (d = dy & 1)
    K = np.empty((128, 65, 66), np.float16)
    for d in range(2):
        K[64*d:64*d+64] = bnpad16[:, d:d+65, :]
    return K

def _build_G(fpad16, half):
    # [128, 34, 66]: G[(d,c), y, x] = fpad[c, r0+y+d, x]
    # (dx-replication into [128,4,34,63] happens on-device)
    r0 = 0 if half == 0 else 31
    G = np.empty((128, 34, 66), np.float16)
    for d in range(2):
        G[64*d:64*d+64] = fpad16[:, r0+d:r0+d+34, :]
    return G

def _make_in_maps(f, b):
    f = np.asarray(f, dtype=np.float32)
    b = np.asarray(b, dtype=np.float32)
    in_maps = []
    for smp in range(4):
        bs = b[smp]
        bn = bs / np.sqrt((bs*bs).sum(axis=(1, 2), keepdims=True) + 1e-8)
        Kmat = _build_K(_pad_edge3(bn).astype(np.float16))
        fpad16 = _pad_edge3(f[smp]).astype(np.float16)
        for half in range(2):
            in_maps.append({
                "g": _build_G(fpad16, half),
                "kn": Kmat,
            })
    return in_maps

def _diag3(x):
    out = x.copy()
    out[:, :, 1:, 1:] += x[:, :, :-1, :-1]
    out[:, :, :-1, :-1] += x[:, :, 1:, 1:]
    return out

def _host_post_full(cos, maskc):
    # cos (B,1024,63,63) fp32, maskc (B,1,64,64) -> softmax out (B,1024,63,63)
    B = cos.shape[0]
    cs, hs, ws = 1024, 63, 63
    hb = wb = 32
    c1 = _diag3(cos.reshape(B, 1, cs, hs*ws))
    c1 = c1.reshape(B, 1, hb, wb, hs, ws).transpose(0, 1, 3, 2, 5, 4).reshape(B, 1, cs, hs*ws)
    c1 = _diag3(np.ascontiguousarray(c1))
    c1 = c1.reshape(B, 1, hb, wb, hs, ws).transpose(0, 1, 3, 2, 5, 4)
    cos = c1.reshape(B, cs, hs, ws)

    # window sums of maskc via integral image (integer-exact in float64)
    mc = np.pad(maskc[:, 0], ((0, 0), (1, 1), (1, 1)), mode='edge').astype(np.float64)
    I = np.zeros((B, 67, 67), np.float64)
    I[:, 1:, 1:] = mc.cumsum(axis=1).cumsum(axis=2)
    S = I[:, 4:, 4:] - I[:, :-4, 4:] - I[:, 4:, :-4] + I[:, :-4, :-4]  # (B,63,63)
    Sp = S                              # stride-1 windows
    Sk = S[:, ::2, ::2]                 # stride-2 windows (32x32)
    mm = (Sk[:, :, :, None, None].reshape(B, cs, 1, 1) > Sp[:, None, :, :])
    ppp = (Sp > 8.0)[:, None, :, :]
    mm = mm & ppp | (Sk.reshape(B, cs, 1, 1) == 16.0)
    cos = cos * mm.astype(np.float32)

    z = cos * 10.0
    z -= z.max(axis=1, keepdims=True)
    np.exp(z, out=z)
    z /= z.sum(axis=1, keepdims=True)
    return z

def kernel(f, b, mask):
    f = np.asarray(f, dtype=np.float32)
    b = np.asarray(b, dtype=np.float32)
    mask = np.asarray(mask, dtype=np.float32)
    B = f.shape[0]
    maskc = 1.0 - mask
    nc = _build_nc()
    in_maps = _make_in_maps(f, b)
    res = bass_utils.run_bass_kernel_spmd(nc, in_maps, list(range(8)))
    cos = np.empty((B, L, 63, 63), np.float32)
    for core in range(8):
        smp, half = core // 2, core % 2
        r0 = 0 if half == 0 else 31
        a = np.asarray(res.results[core]["o"], dtype=np.float32)
        h = a.reshape(NT, 2, 63, L).transpose(3, 0, 1, 2).reshape(L, 32, 63)
        cos[smp, :, r0:r0+32, :] = h
    return _host_post_full(cos, maskc)



# revision 31
# speedup vs baseline: 1.0735x; 1.0735x over previous
"""Trainium2 kernel for nn_CP1_17669495456474 (sparse_attention).
8-core data-parallel: core = (sample, spatial half). Device computes the
grouped cross-correlation (per core: 2016 f-positions x 1024 kernels,
K=1024 contraction) as fp16 tensor-engine matmuls. Contraction is split
as partition=(d, c) with dy=2*dy1+d, so the dy1/dx kernel offsets are
free-dim AP shifts. The kernel rhs slices are pre-arranged contiguously
on host (Kn [128,16,512]) so the PE streams without strided SBUF reads;
G is a single unreplicated [128,34,66] tile sliced per matmul. Host
applies the cheap fuse/mask/softmax."""
import sys, types
import numpy as np

import concourse.bass as bass
import concourse.mybir as mybir
from concourse.tile import TileContext
import concourse.tile as tile_mod
import concourse.bass_utils as bass_utils

F16 = mybir.dt.float16
F32 = mybir.dt.float32
AOT = mybir.AluOpType
NT, TP, L = 16, 126, 1024

# ---------------- compile workarounds (walrus sync-wait limits) ----------------
import orjson

def _patched_drain_and_barrier(self, tick_clock, wait_clock):
    nc = self.nc
    ScopedClock = tile_mod.ScopedClock
    drain_inst = nc.sync.drain()
    wait_clock.add_sem_waits(drain_inst.ins, ScopedClock({None: tick_clock.global_clock}))
    waits = list(drain_inst.ins.sync_info.on_wait)
    if len(waits) > 1:
        import bass_rust
        drain_inst.ins.sync_info = bass_rust.SyncInfo(on_wait=waits[:1], on_update=[])
        for i in range(1, len(waits)):
            d2 = nc.sync.drain()
            d2.ins.sync_info = bass_rust.SyncInfo(on_wait=[waits[i]], on_update=[])
    # Sequencer-level barrier only; skip sem clearing + second barrier.
    # Every run_bass_kernel_spmd call is a fresh NEFF load, which
    # re-initializes semaphores, so recycling them buys nothing and the
    # full teardown (drain butterfly + gpsimd dma_reset + 2nd barrier)
    # costs ~2.5us on the critical path. NOTE: dropping the barrier
    # entirely was tried and wedges the device (engines halting with
    # DMAs in flight) -- keep the sem_only barrier.
    nc.all_engine_barrier(sem_only=True)
    popped = nc._tile_sem_poison_stack.pop()
    assert popped is self._sem_poison

def _dedup_ldweights(m):
    # Drop Ldweights identical to the weights already resident in the PE
    # array (consecutive same-weight loads, e.g. the n=0/n=1 matmul pairs).
    # Sync carried by a dropped load moves to the next PE instruction.
    for f in m.get("functions", []):
        for b in f.get("blocks", []):
            out = []
            last_sig = None
            carry_w, carry_u = [], []
            for inst in b.get("instructions", []):
                if inst.get("engine") == "PE":
                    opc = inst.get("opcode", "")
                    if opc == "Ldweights":
                        sig = orjson.dumps([inst.get("ins"),
                                            inst.get("tile_position"),
                                            inst.get("tile_size"),
                                            inst.get("perf_mode"),
                                            inst.get("is_transpose")])
                        if sig == last_sig:
                            si = inst.get("sync_info") or {}
                            carry_w.extend(si.get("on_wait") or [])
                            carry_u.extend(si.get("on_update") or [])
                            continue
                        last_sig = sig
                    elif opc != "Matmult":
                        last_sig = None
                    if carry_w or carry_u:
                        si = inst.get("sync_info")
                        if si is None:
                            si = {"on_update": [], "on_wait": []}
                            inst["sync_info"] = si
                        si["on_wait"] = carry_w + (si.get("on_wait") or [])
                        si["on_update"] = (si.get("on_update") or []) + carry_u
                        carry_w, carry_u = [], []
                out.append(inst)
            b["instructions"] = out
    return m

def _split_waits_json(bir_bytes):
    m = _dedup_ldweights(orjson.loads(bir_bytes))
    for f in m.get("functions", []):
        for b in f.get("blocks", []):
            insts = b.get("instructions", [])
            out = []
            for inst in insts:
                si = inst.get("sync_info")
                waits = (si or {}).get("on_wait") or []
                opc = inst.get("opcode", "")
                is_dma = opc.startswith("DMA") or "Trigger" in opc or "Dma" in opc
                keep = 1
                if is_dma and len(waits) <= 1:
                    out.append(inst)
                    continue
                if len(waits) > keep:
                    si["on_wait"] = waits[-keep:]
                    for i, w in enumerate(waits[:-keep]):
                        out.append({
                            "debug": inst.get("debug", 0), "engine": inst["engine"],
                            "ins": [], "outs": [], "name": f"{inst['name']}_xw{i}",
                            "opcode": "EventSemaphore",
                            "sync_info": {"on_update": [], "on_wait": [w]},
                        })
                out.append(inst)
            b["instructions"] = out
    return orjson.dumps(m)

def _install_patches():
    if getattr(bass_utils.compile_bir_kernel, "_wait_split", False):
        return
    TileContext._drain_and_barrier = _patched_drain_and_barrier
    import concourse.bass2jax as b2j
    orig = bass_utils.compile_bir_kernel
    def wrapped(bir_str, *a, **kw):
        if isinstance(bir_str, (bytes, bytearray)):
            try:
                bir_str = _split_waits_json(bir_str)
            except Exception:
                pass
        return orig(bir_str, *a, **kw)
    wrapped._wait_split = True
    bass_utils.compile_bir_kernel = wrapped
    if hasattr(b2j, "compile_bir_kernel"):
        b2j.compile_bir_kernel = wrapped
    # NTFF hook shim so trace=True doesn't crash if requested elsewhere
    if "antenv.axon_hooks" not in sys.modules:
        mod = types.ModuleType("antenv.axon_hooks")
        mod._hook = None
        mod.set_axon_ntff_profile_hook = lambda h: setattr(mod, "_hook", h)
        mod.get_axon_ntff_profile_hook = lambda: mod._hook
        sys.modules["antenv.axon_hooks"] = mod
        try:
            from trn_agent_boot.trn_boot import _ntff_profile_via_ctypes
            hk = _ntff_profile_via_ctypes('/opt/axon/libaxon_pjrt.so')
            if hk is not None:
                mod._hook = hk
        except Exception:
            pass
        bass_utils.upload_artifacts = lambda tmpdir: str(tmpdir)

# ---------------- device program: raw cos in [p, l] tiles ----------------
_NC_CACHE = [None]

def _build_nc():
    if _NC_CACHE[0] is not None:
        return _NC_CACHE[0]
    _install_patches()
    nc = bass.Bass("TRN2", target_bir_lowering=False, debug=False)
    g_d = nc.dram_tensor("g", [128, 34, 66], F16, kind="ExternalInput")
    K_d = nc.dram_tensor("kn", [128, 16, 512], F16, kind="ExternalInput")
    o_d = nc.dram_tensor("o", [NT, TP, L], F16, kind="ExternalOutput")
    with TileContext(nc) as tc:
        import contextlib
        ctx = contextlib.ExitStack()
        with ctx:
            const = ctx.enter_context(tc.tile_pool(name="const", bufs=1))
            outp = ctx.enter_context(tc.tile_pool(name="outp", bufs=3))
            psp = ctx.enter_context(tc.tile_pool(name="psp", bufs=3, space="PSUM"))
            psw = ctx.enter_context(tc.tile_pool(name="psw", bufs=1, space="PSUM"))
            G1 = const.tile([128, 34, 66], F16, tag="G1")
            G4 = const.tile([128, 4, 34, 63], F16, tag="G4")
            Kn = const.tile([128, 16, 512], F16, tag="Kn")
            Wd = const.tile([128, 126], F16, tag="Wd")
            Xd = const.tile([128, 512], F16, tag="Xd")
            # warmup: ramp the PE p-state while the inputs land
            nc.vector.memset(Wd[:], 0.0)
            nc.gpsimd.memset(Xd[:], 0.0)
            pw = psw.tile([128, 512], F32, tag="pw")
            for _ in range(14):
                nc.tensor.matmul(pw[0:TP, :], Wd[:], Xd[:], start=True, stop=True,
                                 skip_group_check=True)
            # Input waves. Tile 0 (n-first) consumes Kn[m] sequentially at
            # 593GB/s -- faster than DMA supplies it -- so after G1's first
            # rows, Kn gets the full DMA bandwidth. G loads compact (574KB
            # instead of 2.45MB) and is dx-replicated on-device by the
            # otherwise-idle engines (nearly-contiguous reads; a stride-2
            # on-device Kn rearrange was tried and is 4-5x slower per
            # element, starving the PE -- Kn's redundancy ships via DMA).
            engs = [nc.scalar, nc.sync, nc.gpsimd]
            q = 0
            wave = [(G1[:, 0:8], g_d[:, 0:8])]
            for m in range(16):
                wave.append((Kn[:, m], K_d[:, m]))
            wave.append((G1[:, 8:21], g_d[:, 8:21]))
            wave.append((G1[:, 21:34], g_d[:, 21:34]))
            for dst, src in wave:
                engs[q % 3].dma_start(out=dst, in_=src)
                q += 1
            # dx-replication: G4[:, dx, ys, :] = G1[:, ys, dx:dx+63]
            ceng = [nc.vector, nc.gpsimd, nc.vector, nc.scalar]
            for ci, ys in enumerate((slice(0, 8), slice(8, 21), slice(21, 34))):
                for dx in range(4):
                    e = ceng[(ci * 4 + dx) % 4]
                    if e is nc.scalar:
                        e.copy(out=G4[:, dx, ys, :], in_=G1[:, ys, dx:dx+63])
                    else:
                        e.tensor_copy(G4[:, dx, ys, :], G1[:, ys, dx:dx+63])

            for t in range(NT):
                ps0 = psp.tile([128, 512], F32, tag="ps0", name="ps0")
                ps1 = psp.tile([128, 512], F32, tag="ps1", name="ps1")
                pss = (ps0, ps1)
                if t == 0 or t == NT - 1:
                    # n-first. Tile 0: the first 8 matmuls need only Kn[0:8],
                    # which lands earlier. Last tile: the n=0 PSUM bank
                    # finishes at matmul 8, letting its copy+DMA overlap the
                    # final 8 matmuls.
                    order = [(dy1, dx, n) for n in range(2) for dy1 in range(2)
                             for dx in range(4)]
                    starts = {0, 8}
                    stops = {7, 15}
                else:
                    order = [(dy1, dx, n) for dy1 in range(2) for dx in range(4)
                             for n in range(2)]
                    starts = {0, 1}
                    stops = {14, 15}
                for kk, (dy1, dx, n) in enumerate(order):
                    y0 = 2*t + 2*dy1
                    lhsT = G4[:, dx, y0:y0+2, :]
                    rhs = Kn[:, 8*n + 4*dy1 + dx]
                    nc.tensor.matmul(pss[n][0:TP, :], lhsT, rhs,
                                     start=(kk in starts), stop=(kk in stops),
                                     skip_group_check=True)
                O = outp.tile([128, 1024], F16, tag="O", name="O")
                if t == NT - 1:
                    # chunked copies + prompt DMA so the n=0 half streams out
                    # while the n=1 matmuls still run; keep gpsimd out of the
                    # final transfers so its (slowest) teardown drain starts
                    # as early as possible
                    nc.scalar.copy(out=O[0:TP, 0:256], in_=ps0[0:TP, 0:256])
                    nc.vector.tensor_copy(O[0:TP, 256:512], ps0[0:TP, 256:512])
                    nc.scalar.dma_start(out=o_d[t, :, 0:256], in_=O[0:TP, 0:256])
                    nc.sync.dma_start(out=o_d[t, :, 256:512], in_=O[0:TP, 256:512])
                    nc.scalar.copy(out=O[0:TP, 512:768], in_=ps1[0:TP, 0:256])
                    nc.vector.tensor_copy(O[0:TP, 768:1024], ps1[0:TP, 256:512])
                    nc.scalar.dma_start(out=o_d[t, :, 512:768], in_=O[0:TP, 512:768])
                    nc.sync.dma_start(out=o_d[t, :, 768:1024], in_=O[0:TP, 768:1024])
                else:
                    nc.scalar.copy(out=O[0:TP, 0:512], in_=ps0[0:TP, :])
                    nc.vector.tensor_copy(O[0:TP, 512:1024], ps1[0:TP, :])
                    e0, e1 = (nc.sync, nc.gpsimd) if t % 2 == 0 else (nc.gpsimd, nc.sync)
                    e0.dma_start(out=o_d[t, :, 0:512], in_=O[0:TP, 0:512])
                    e1.dma_start(out=o_d[t, :, 512:1024], in_=O[0:TP, 512:1024])
    _NC_CACHE[0] = nc
    return nc

# ---------------- host side ----------------
def _pad_edge3(x):
    return np.pad(x, ((0, 0), (1, 1), (1, 1)), mode='edge')

def _build_Kn(bnpad16):
    # [128, 16, 512]: Kn[(d,c), 8n+4dy1+dx, 32j+i] = bnpad[c, 32n+2dy1+2j+d, dx+2i]
    # i.e. each matmul's rhs slice, pre-arranged contiguously.
    Kn = np.empty((128, 16, 512), np.float16)
    for d in range(2):
        for n in range(2):
            for dy1 in range(2):
                for dx in range(4):
                    r = 32*n + 2*dy1 + d
                    blk = bnpad16[:, r:r+31:2, dx:dx+63:2]       # (64, 16, 32)
                    Kn[64*d:64*d+64, 8*n + 4*dy1 + dx] = blk.reshape(64, 512)
    return Kn

def _build_G(fpad16, half):
    # [128, 34, 66]: G[(d,c), y, x] = fpad[c, r0+y+d, x]
    # (dx-replication into [128,4,34,63] happens on-device)
    r0 = 0 if half == 0 else 31
    G = np.empty((128, 34, 66), np.float16)
    for d in range(2):
        G[64*d:64*d+64] = fpad16[:, r0+d:r0+d+34, :]
    return G

def _make_in_maps(f, b):
    f = np.asarray(f, dtype=np.float32)
    b = np.asarray(b, dtype=np.float32)
    in_maps = []
    for smp in range(4):
        bs = b[smp]
        bn = bs / np.sqrt((bs*bs).sum(axis=(1, 2), keepdims=True) + 1e-8)
        Kn = _build_Kn(_pad_edge3(bn).astype(np.float16))
        fpad16 = _pad_edge3(f[smp]).astype(np.float16)
        for half in range(2):
            in_maps.append({
                "g": _build_G(fpad16, half),
                "kn": Kn,
            })
    return in_maps

def _diag3(x):
    out = x.copy()
    out[:, :, 1:, 1:] += x[:, :, :-1, :-1]
    out[:, :, :-1, :-1] += x[:, :, 1:, 1:]
    return out

def _host_post_full(cos, maskc):
    # cos (B,1024,63,63) fp32, maskc (B,1,64,64) -> softmax out (B,1024,63,63)
    B = cos.shape[0]
    cs, hs, ws = 1024, 63, 63
    hb = wb = 32
    c1 = _diag3(cos.reshape(B, 1, cs, hs*ws))
    c1 = c1.reshape(B, 1, hb, wb, hs, ws).transpose(0, 1, 3, 2, 5, 4).reshape(B, 1, cs, hs*ws)
    c1 = _diag3(np.ascontiguousarray(c1))
    c1 = c1.reshape(B, 1, hb, wb, hs, ws).transpose(0, 1, 3, 2, 5, 4)
    cos = c1.reshape(B, cs, hs, ws)

    # window sums of maskc via integral image (integer-exact in float64)
    mc = np.pad(maskc[:, 0], ((0, 0), (1, 1), (1, 1)), mode='edge').astype(np.float64)
    I = np.zeros((B, 67, 67), np.float64)
    I[:, 1:, 1:] = mc.cumsum(axis=1).cumsum(axis=2)
    S = I[:, 4:, 4:] - I[:, :-4, 4:] - I[:, 4:, :-4] + I[:, :-4, :-4]  # (B,63,63)
    Sp = S                              # stride-1 windows
    Sk = S[:, ::2, ::2]                 # stride-2 windows (32x32)
    mm = (Sk[:, :, :, None, None].reshape(B, cs, 1, 1) > Sp[:, None, :, :])
    ppp = (Sp > 8.0)[:, None, :, :]
    mm = mm & ppp | (Sk.reshape(B, cs, 1, 1) == 16.0)
    cos = cos * mm.astype(np.float32)

    z = cos * 10.0
    z -= z.max(axis=1, keepdims=True)
    np.exp(z, out=z)
    z /= z.sum(axis=1, keepdims=True)
    return z

def kernel(f, b, mask):
    f = np.asarray(f, dtype=np.float32)
    b = np.asarray(b, dtype=np.float32)
    mask = np.asarray(mask, dtype=np.float32)
    B = f.shape[0]
    maskc = 1.0 - mask
    nc = _build_nc()
    in_maps = _make_in_maps(f, b)
    res = bass_utils.run_bass_kernel_spmd(nc, in_maps, list(range(8)))
    cos = np.empty((B, L, 63, 63), np.float32)
    for core in range(8):
        smp, half = core // 2, core % 2
        r0 = 0 if half == 0 else 31
        a = np.asarray(res.results[core]["o"], dtype=np.float32)
        h = a.reshape(NT, 2, 63, L).transpose(3, 0, 1, 2).reshape(L, 32, 63)
        cos[smp, :, r0:r0+32, :] = h
    return _host_post_full(cos, maskc)

